# revision 8
# baseline (speedup 1.0000x reference)
"""Multi-head causal self-attention (B=4, T=2048, D=1024, H=16) on 8 TRN2 cores.

Sharding (hardcoded): data-parallel over the 4 batches x tensor-parallel over
head halves. Core c handles batch c//2 and local heads (c%2)*8 .. (c%2)*8+7
for all 2048 positions. Every core runs the same SPMD program on its slice:

  x[b] [2048,1024] -> bf16 -> x^T in SBUF via SBUF->SBUF DMA transposes
  Q^T = (Wq_slice)^T x^T,  K^T = (Wk_slice)^T x^T  (dh-pairs packed on 128
  partitions), V = x Wv_slice (+ ones column for the softmax denominator)
  S^T = K Q^T per 128x512 block; the two heads of a pair run as concurrent
  row-group matmuls (tile_position (0,0)/(64,0)) into one [128,1024] psum
  tile; diagonal blocks stream only the causal columns,
  P^T = exp(S^T / 8) (one ScalarE activation per key block covering both
  parities, bf16 out), strict-upper within-block masking via utri mul,
  ctx^T = V^T P per 4-key-block segment (V stationary; row 64 = denom l),
  normalize: 1/l broadcast via a K=2 selector matmul on the PE,
  partial_out = ctx^T.T @ Wo_slice.

The host sums the two partial outputs per batch and adds the bias bo.
"""
import ml_dtypes
import numpy as np

import concourse.bass as bass
import concourse.mybir as mybir
import concourse.tile as tile
from concourse import bacc
from concourse.bass_utils import run_bass_kernel_spmd
from concourse.masks import make_upper_triangular

F32 = mybir.dt.float32
BF16 = mybir.dt.bfloat16
AF = mybir.ActivationFunctionType

B, T, D = 4, 2048, 1024
HL = 8              # local heads per core
HP = HL // 2        # local head pairs (two heads share 128 partitions)
DH = 64
PO = D // 128       # contraction chunks over D
CD = HL * DH        # 512: local context feature dim
FC = CD // 128      # 4
NB = T // 128       # 16 query/key blocks of 128
QUAD = 4            # query blocks handled together (512 S^T columns)
SCALE = 1.0 / 8.0   # 1/sqrt(DH)
SEG = 4             # key blocks per P^T segment (AV granularity)


def _emit_xt_tb(nc, tb, x_d, xstage, x16p, xt_sb):
    """One 128-row block of x: f32 load, bf16 cast (GpSimd), 8 SBUF->SBUF
    DMA transposes spread over the two hwdge queues."""
    xf = xstage.tile([128, D], F32, tag="xf")
    nc.scalar.dma_start(xf, x_d[tb * 128:(tb + 1) * 128, :])
    x16 = x16p.tile([128, D], BF16, tag="x16")
    nc.gpsimd.tensor_copy(x16, xf)
    for po in range(PO):
        eng = nc.sync if po % 2 == 0 else nc.scalar
        eng.dma_start_transpose(
            xt_sb[:, po, tb * 128:(tb + 1) * 128],
            x16[:, po * 128:(po + 1) * 128])


def _emit_proj_tw(nc, tw, xt_sb, mmp, wq_sb, wk_sb, wv_sb, kt_sb, qt_sb, v_sb):
    """Project K^T, Q^T, V for one 512-column group of x^T."""
    tsl = slice(tw * 512, (tw + 1) * 512)
    for hp in range(HP):
        ps = mmp.tile([128, 512], F32, tag="mm")
        for po in range(PO):
            nc.tensor.matmul(
                ps, lhsT=wk_sb[:, po, hp * 128:(hp + 1) * 128], rhs=xt_sb[:, po, tsl],
                start=(po == 0), stop=(po == PO - 1),
            )
        nc.vector.tensor_copy(kt_sb[:, hp, tsl], ps)
    for hp in range(HP):
        ps = mmp.tile([128, 512], F32, tag="mm")
        for po in range(PO):
            nc.tensor.matmul(
                ps, lhsT=wq_sb[:, po, hp * 128:(hp + 1) * 128], rhs=xt_sb[:, po, tsl],
                start=(po == 0), stop=(po == PO - 1),
            )
        nc.vector.tensor_copy(qt_sb[:, hp, tsl], ps)
    for tb in range(4):
        kb = tw * 4 + tb
        ps = mmp.tile([128, 512], F32, tag="mm")
        for po in range(PO):
            nc.tensor.matmul(
                ps, lhsT=xt_sb[:, po, kb * 128:(kb + 1) * 128], rhs=wv_sb[:, po, :],
                start=(po == 0), stop=(po == PO - 1),
            )
        nc.vector.tensor_copy(
            v_sb[:, kb, :, 0:64], ps.rearrange("p (h d) -> p h d", h=HL)
        )


def _emit_attention_quad(nc, qb0, kt_sb, qt_sb, v_sb, utri01, sel2,
                         ptp, stp, cxp, lvp, mmp, ctxt16s):
    """Attention for query blocks qb0..qb0+3, all 4 local head pairs.

    For each key block kb one matmul pair (row groups (0,0)/(64,0), shared
    [128,1024] psum tile) covers all four query blocks of both parities.
    Diagonal key blocks stream only columns >= the diagonal; P^T columns
    left of the diagonal are never written nor read.
    """
    nkb = qb0 + QUAD
    nseg = (nkb + SEG - 1) // SEG
    for hp in range(HP):
        ctxt16 = ctxt16s[hp]
        psA = cxp.tile([128, 512], F32, tag="cxA")
        psB = cxp.tile([128, 512], F32, tag="cxB")
        for seg in range(nseg):
            s0, s1 = seg * SEG, min(nkb, (seg + 1) * SEG)
            pt = ptp.tile([128, SEG, 2, 512], BF16, tag="pt")
            for kb in range(s0, s1):
                st = stp.tile([128, 1024], F32, tag="st")
                j = max(0, kb - qb0)  # first valid 128-col of the quad
                for par, tp in ((0, (0, 0)), (1, (64, 0))):
                    lo = 64 * par
                    nc.tensor.matmul(
                        st[:, par * 512 + j * 128:(par + 1) * 512],
                        lhsT=kt_sb[lo:lo + 64, hp, kb * 128:(kb + 1) * 128],
                        rhs=qt_sb[lo:lo + 64, hp,
                                  (qb0 + j) * 128:(qb0 + QUAD) * 128],
                        start=True, stop=True, tile_position=tp,
                    )
                # exp of both parities in one ScalarE instruction, only the
                # causal columns of the quad
                st_v = st.rearrange("p (a c) -> p a c", a=2)
                nc.scalar.activation(pt[:, kb - s0, :, j * 128:512],
                                     st_v[:, :, j * 128:512], AF.Exp,
                                     scale=SCALE)
                if kb >= qb0:
                    for par in range(2):
                        nc.vector.tensor_mul(
                            pt[:, kb - s0, par, j * 128:(j + 1) * 128],
                            pt[:, kb - s0, par, j * 128:(j + 1) * 128], utri01)
            # AV for this segment (accumulating across segments)
            for par, ps in ((0, psA), (1, psB)):
                for kb in range(s0, s1):
                    j = max(0, kb - qb0)
                    nc.tensor.matmul(
                        ps[0:65, j * 128:512],
                        lhsT=v_sb[:, kb, 2 * hp + par, :],
                        rhs=pt[:, kb - s0, par, j * 128:512],
                        start=(kb == 0), stop=(kb == nkb - 1),
                    )
        # normalize: ctx^T[dh, q] /= l[q] with l = psum row 64
        cA = lvp.tile([65, 512], BF16, tag="cA")
        nc.vector.tensor_copy(cA, psA[0:65, :])
        cB = lvp.tile([65, 512], BF16, tag="cB")
        nc.vector.tensor_copy(cB, psB[0:65, :])
        # par1 ctx to partitions 64:128 (DMA partition shift, early issue)
        nc.scalar.dma_start(ctxt16[64:128, :], cB[0:64, :])
        ll = lvp.tile([2, 512], BF16, tag="ll")
        nc.sync.dma_start(ll[0:1, :], cA[64:65, :])
        nc.sync.dma_start(ll[1:2, :], cB[64:65, :])
        lli = lvp.tile([2, 512], F32, tag="lli")
        nc.vector.reciprocal(lli, ll)
        lli16 = lvp.tile([2, 512], BF16, tag="lli16")
        nc.vector.tensor_copy(lli16, lli)
        bc = mmp.tile([128, 512], F32, tag="mm")
        nc.tensor.matmul(bc, lhsT=sel2, rhs=lli16, start=True, stop=True)
        nc.vector.tensor_mul(ctxt16[0:64, :], cA[0:64, :], bc[0:64, :])
        nc.vector.tensor_mul(ctxt16[64:128, :], ctxt16[64:128, :],
                             bc[64:128, :])


def _emit_out_proj_qb(nc, qb, ctxt16s, wo_sb, mmp, osbp, out_d):
    """Output projection for query block qb (reads all 4 head pairs)."""
    qloc = qb % QUAD
    for dw in range(2):
        ps = mmp.tile([128, 512], F32, tag="mm")
        for hp in range(HP):
            nc.tensor.matmul(
                ps, lhsT=ctxt16s[hp][:, qloc * 128:(qloc + 1) * 128],
                rhs=wo_sb[:, hp, dw * 512:(dw + 1) * 512],
                start=(hp == 0), stop=(hp == HP - 1),
            )
        osb = osbp.tile([128, 512], F32, tag="osb")
        nc.vector.tensor_copy(osb, ps)
        eng = nc.sync if dw == 0 else nc.scalar
        eng.dma_start(out_d[qb * 128:(qb + 1) * 128, dw * 512:(dw + 1) * 512], osb)


def build_nc():
    nc = bacc.Bacc("TRN2", target_bir_lowering=False)
    x_d = nc.dram_tensor("x", [T, D], F32, kind="ExternalInput")
    wq_d = nc.dram_tensor("wq", [D, CD], F32, kind="ExternalInput")
    wk_d = nc.dram_tensor("wk", [D, CD], F32, kind="ExternalInput")
    wv_d = nc.dram_tensor("wv", [D, CD], F32, kind="ExternalInput")
    wo_d = nc.dram_tensor("wo", [CD, D], F32, kind="ExternalInput")
    sel_d = nc.dram_tensor("sel", [2, 128], BF16, kind="ExternalInput")
    out_d = nc.dram_tensor("out", [T, D], F32, kind="ExternalOutput")

    with tile.TileContext(nc) as tc:
        with (
            tc.tile_pool(name="consts", bufs=1) as consts,
            tc.tile_pool(name="wsb", bufs=1) as wsb,
            tc.tile_pool(name="wstage", bufs=1) as wstage,
            tc.tile_pool(name="xstage", bufs=2) as xstage,
            tc.tile_pool(name="x16", bufs=2) as x16p,
            tc.tile_pool(name="big", bufs=1) as big,
            tc.tile_pool(name="pt", bufs=2) as ptp,
            tc.tile_pool(name="lv", bufs=2) as lvp,
            tc.tile_pool(name="ctxt16", bufs=8) as ctxt16p,
            tc.tile_pool(name="osb", bufs=2) as osbp,
            tc.tile_pool(name="mm", bufs=2, space="PSUM") as mmp,
            tc.tile_pool(name="st", bufs=2, space="PSUM") as stp,
            tc.tile_pool(name="cx", bufs=1, space="PSUM") as cxp,
        ):
            utri01 = consts.tile([128, 128], BF16, tag="utri01")
            make_upper_triangular(nc, utri01, val=1.0, diag=True)
            sel2 = consts.tile([2, 128], BF16, tag="sel2")
            nc.sync.dma_start(sel2, sel_d[:, :])

            wq_sb = wsb.tile([128, PO, CD], BF16, tag="wq")
            wk_sb = wsb.tile([128, PO, CD], BF16, tag="wk")
            wv_sb = wsb.tile([128, PO, CD], BF16, tag="wv")
            wo_sb = wsb.tile([128, FC, D], BF16, tag="wo")
            for i, (dram, sb, shp) in enumerate((
                (wk_d, wk_sb, (PO, CD)),
                (wq_d, wq_sb, (PO, CD)),
                (wv_d, wv_sb, (PO, CD)),
                (wo_d, wo_sb, (FC, D)),
            )):
                stg = wstage.tile([128, shp[0], shp[1]], F32, tag="ws")
                nc.sync.dma_start(stg, dram.rearrange("(po p) n -> p po n", p=128))
                nc.gpsimd.tensor_copy(sb, stg)

            xt_sb = big.tile([128, PO, T], BF16, tag="xt")
            kt_sb = big.tile([128, HP, T], BF16, tag="kt")
            qt_sb = big.tile([128, HP, T], BF16, tag="qt")
            v_sb = big.tile([128, NB, HL, 65], BF16, tag="v")
            nc.gpsimd.memset(v_sb[:, :, :, 64:65], 1.0)

            for tb in range(NB):
                _emit_xt_tb(nc, tb, x_d, xstage, x16p, xt_sb)

            for tw in range(4):
                _emit_proj_tw(nc, tw, xt_sb, mmp,
                              wq_sb, wk_sb, wv_sb, kt_sb, qt_sb, v_sb)
                qb0 = 4 * tw
                ctxt16s = [ctxt16p.tile([128, 512], BF16, tag="c16",
                                        name=f"c16_{hp}")
                           for hp in range(HP)]
                _emit_attention_quad(nc, qb0, kt_sb, qt_sb, v_sb, utri01,
                                     sel2, ptp, stp, cxp, lvp, mmp, ctxt16s)
                for qloc in range(QUAD):
                    _emit_out_proj_qb(nc, qb0 + qloc, ctxt16s, wo_sb,
                                      mmp, osbp, out_d)

    nc.compile()
    return nc


_CACHE = {}


def _get_nc():
    if "nc" not in _CACHE:
        _CACHE["nc"] = build_nc()
    return _CACHE["nc"]


def make_in_maps(x, Wq, Wk, Wv, Wo):
    x = np.asarray(x, np.float32)
    Wq = np.asarray(Wq, np.float32)
    Wk = np.asarray(Wk, np.float32)
    Wv = np.asarray(Wv, np.float32)
    Wo = np.asarray(Wo, np.float32)
    sel = np.zeros((2, 128), np.float32)
    sel[0, 0:64] = 1.0
    sel[1, 64:128] = 1.0
    sel = sel.astype(ml_dtypes.bfloat16)
    in_maps = []
    for c in range(8):
        b, hh = c // 2, c % 2
        cols = slice(hh * CD, (hh + 1) * CD)
        in_maps.append({
            "x": np.ascontiguousarray(x[b]),
            "wq": np.ascontiguousarray(Wq[:, cols]),
            "wk": np.ascontiguousarray(Wk[:, cols]),
            "wv": np.ascontiguousarray(Wv[:, cols]),
            "wo": np.ascontiguousarray(Wo[cols, :]),
            "sel": sel,
        })
    return in_maps


def gather_output(results, bo):
    bo = np.asarray(bo, np.float32)
    out = np.empty((B, T, D), np.float32)
    for b in range(B):
        out[b] = results[2 * b]["out"] + results[2 * b + 1]["out"] + bo[None, :]
    return out


def kernel(x, Wq, Wk, Wv, Wo, bo):
    nc = _get_nc()
    in_maps = make_in_maps(x, Wq, Wk, Wv, Wo)
    res = run_bass_kernel_spmd(nc, in_maps, core_ids=list(range(8)))
    return gather_output(res.results, bo)


# revision 17
# speedup vs baseline: 1.2114x; 1.2114x over previous
"""Multi-head causal self-attention (B=4, T=2048, D=1024, H=16) on 8 TRN2 cores.

Sharding (hardcoded): data-parallel over the 4 batches x tensor-parallel over
head halves. Core c handles batch c//2 and local heads (c%2)*8 .. (c%2)*8+7
for all 2048 positions. Every core runs the same SPMD program on its slice:

  x[b] [2048,1024] -> bf16 -> x^T in SBUF via SBUF->SBUF DMA transposes
  Q^T = (Wq_slice)^T x^T,  K^T = (Wk_slice)^T x^T  (dh-pairs packed on 128
  partitions), V = x Wv_slice (+ ones column for the softmax denominator)
  S^T = K Q^T per 128x512 block; the two heads of a pair run as concurrent
  row-group matmuls (tile_position (0,0)/(64,0)) into one [128,1024] psum
  tile; diagonal blocks stream only the causal columns,
  P^T = exp(S^T / 8) (one ScalarE activation per key block covering both
  parities, bf16 out), strict-upper within-block masking via utri mul,
  ctx^T = V^T P per 4-key-block segment (V stationary; row 64 = denom l),
  normalize: 1/l broadcast via a K=2 selector matmul on the PE,
  partial_out = ctx^T.T @ Wo_slice.

The host sums the two partial outputs per batch and adds the bias bo.
"""
import ml_dtypes
import numpy as np

import concourse.bass as bass
import concourse.mybir as mybir
import concourse.tile as tile
from concourse import bacc
from concourse.bass_utils import run_bass_kernel_spmd
from concourse.masks import make_upper_triangular

F32 = mybir.dt.float32
BF16 = mybir.dt.bfloat16
AF = mybir.ActivationFunctionType

B, T, D = 4, 2048, 1024
HL = 8              # local heads per core
HP = HL // 2        # local head pairs (two heads share 128 partitions)
DH = 64
PO = D // 128       # contraction chunks over D
CD = HL * DH        # 512: local context feature dim
FC = CD // 128      # 4
NB = T // 128       # 16 query/key blocks of 128
QUAD = 4            # query blocks handled together (512 S^T columns)
SCALE = 1.0 / 8.0   # 1/sqrt(DH)
SEG = 4             # key blocks per P^T segment (AV granularity)


def _emit_xt_tb(nc, tb, x_d, xb16_d, xstage, x16p):
    """One 128-row block of x: f32 load (scalar q), bf16 cast (DVE), store
    to DRAM scratch (gpsimd swdge q)."""
    xf = xstage.tile([128, D], F32, tag="xf")
    nc.scalar.dma_start(xf, x_d[tb * 128:(tb + 1) * 128, :])
    x16 = x16p.tile([128, D], BF16, tag="x16")
    nc.vector.tensor_copy(x16, xf)
    nc.gpsimd.dma_start(xb16_d[tb * 128:(tb + 1) * 128, :], x16)


def _emit_proj_tw(nc, tw, xt_sb, mmp, wq_sb, wk_sb, wv_sb, kt_sb, qt_sb, v_sb):
    """Project K^T, Q^T, V for one 512-column group of x^T."""
    tsl = slice(tw * 512, (tw + 1) * 512)
    for hp in range(HP):
        ps = mmp.tile([128, 512], F32, tag="mm")
        for po in range(PO):
            nc.tensor.matmul(
                ps, lhsT=wk_sb[:, po, hp * 128:(hp + 1) * 128], rhs=xt_sb[:, po, tsl],
                start=(po == 0), stop=(po == PO - 1),
            )
        nc.vector.tensor_copy(kt_sb[:, hp, tsl], ps)
    for hp in range(HP):
        ps = mmp.tile([128, 512], F32, tag="mm")
        for po in range(PO):
            nc.tensor.matmul(
                ps, lhsT=wq_sb[:, po, hp * 128:(hp + 1) * 128], rhs=xt_sb[:, po, tsl],
                start=(po == 0), stop=(po == PO - 1),
            )
        nc.vector.tensor_copy(qt_sb[:, hp, tsl], ps)
    for tb in range(4):
        kb = tw * 4 + tb
        ps = mmp.tile([128, 512], F32, tag="mm")
        for po in range(PO):
            nc.tensor.matmul(
                ps, lhsT=xt_sb[:, po, kb * 128:(kb + 1) * 128], rhs=wv_sb[:, po, :],
                start=(po == 0), stop=(po == PO - 1),
            )
        nc.vector.tensor_copy(
            v_sb[:, kb, :, 0:64], ps.rearrange("p (h d) -> p h d", h=HL)
        )


def _emit_attention_quad(nc, qb0, kt_sb, qt_sb, v_sb, utri01, sel2,
                         ptp, stp, cxp, lvp, mmp, ctxt16s):
    """Attention for query blocks qb0..qb0+3, all 4 local head pairs.

    For each key block kb one matmul pair (row groups (0,0)/(64,0), shared
    [128,1024] psum tile) covers all four query blocks of both parities.
    Diagonal key blocks stream only columns >= the diagonal; P^T columns
    left of the diagonal are never written nor read.
    """
    nkb = qb0 + QUAD
    nseg = (nkb + SEG - 1) // SEG
    # per-quad softmax denominators: l rows gathered as [64, (hp, par, 8)]
    # so the reciprocal runs 16 elems/lane instead of 512
    lv = lvp.tile([64, HP, 2, 8], BF16, tag="lv")
    cAs, cBs = [], []
    for hp in range(HP):
        ctxt16 = ctxt16s[hp]
        psA = cxp.tile([128, 512], F32, tag="cxA")
        psB = cxp.tile([128, 512], F32, tag="cxB")
        for seg in range(nseg):
            s0, s1 = seg * SEG, min(nkb, (seg + 1) * SEG)
            pt = ptp.tile([128, SEG, 2, 512], BF16, tag="pt")
            for kb in range(s0, s1):
                st = stp.tile([128, 1024], F32, tag="st")
                j = max(0, kb - qb0)  # first valid 128-col of the quad
                for par, tp in ((0, (0, 0)), (1, (64, 0))):
                    lo = 64 * par
                    nc.tensor.matmul(
                        st[:, par * 512 + j * 128:(par + 1) * 512],
                        lhsT=kt_sb[lo:lo + 64, hp, kb * 128:(kb + 1) * 128],
                        rhs=qt_sb[lo:lo + 64, hp,
                                  (qb0 + j) * 128:(qb0 + QUAD) * 128],
                        start=True, stop=True, tile_position=tp,
                    )
                # exp of both parities in one ScalarE instruction, only the
                # causal columns of the quad
                st_v = st.rearrange("p (a c) -> p a c", a=2)
                nc.scalar.activation(pt[:, kb - s0, :, j * 128:512],
                                     st_v[:, :, j * 128:512], AF.Exp,
                                     scale=SCALE)
                if kb >= qb0:
                    for par in range(2):
                        nc.vector.tensor_mul(
                            pt[:, kb - s0, par, j * 128:(j + 1) * 128],
                            pt[:, kb - s0, par, j * 128:(j + 1) * 128], utri01)
            # AV for this segment (accumulating across segments)
            for par, ps in ((0, psA), (1, psB)):
                for kb in range(s0, s1):
                    j = max(0, kb - qb0)
                    nc.tensor.matmul(
                        ps[0:65, j * 128:512],
                        lhsT=v_sb[:, kb, 2 * hp + par, :],
                        rhs=pt[:, kb - s0, par, j * 128:512],
                        start=(kb == 0), stop=(kb == nkb - 1),
                    )
        # psum -> sbuf copies (free the cx banks), l rows into the quad
        # gather, par1 ctx partition-shifted to 64:128 early
        cA = lvp.tile([65, 512], BF16, tag="cA", name=f"cA{hp}")
        nc.vector.tensor_copy(cA, psA[0:65, :])
        cB = lvp.tile([65, 512], BF16, tag="cB", name=f"cB{hp}")
        nc.vector.tensor_copy(cB, psB[0:65, :])
        nc.sync.dma_start(ctxt16[64:128, :], cB[0:64, :])
        nc.sync.dma_start(lv[:, hp, 0, :], cA[64:65, :])
        nc.sync.dma_start(lv[:, hp, 1, :], cB[64:65, :])
        cAs.append(cA)
        cBs.append(cB)
    # one reciprocal for the whole quad (16 elems/lane), scatter to the
    # [2, hp, q] layout the selector matmul streams from
    lvi = lvp.tile([64, HP, 2, 8], F32, tag="lvi")
    nc.vector.reciprocal(lvi, lv)
    lvi16 = lvp.tile([64, HP, 2, 8], BF16, tag="lvi16")
    nc.vector.tensor_copy(lvi16, lvi)
    llin = lvp.tile([2, HP, 64, 8], BF16, tag="llin")
    for par in range(2):
        for h in range(HP):
            nc.sync.dma_start(llin[par:par + 1, h, :, :],
                              lvi16[:, h, par, :])
    for hp in range(HP):
        ctxt16 = ctxt16s[hp]
        bc = mmp.tile([128, 512], F32, tag="mm")
        nc.tensor.matmul(bc, lhsT=sel2, rhs=llin[:, hp, :, :],
                         start=True, stop=True)
        nc.vector.tensor_mul(ctxt16[0:64, :], cAs[hp][0:64, :], bc[0:64, :])
        nc.vector.tensor_mul(ctxt16[64:128, :], ctxt16[64:128, :],
                             bc[64:128, :])


def _emit_out_proj_qb(nc, qb, ctxt16s, wo_sb, mmp, osbp, out_d):
    """Output projection for query block qb (reads all 4 head pairs)."""
    qloc = qb % QUAD
    for dw in range(2):
        ps = mmp.tile([128, 512], F32, tag="mm")
        for hp in range(HP):
            nc.tensor.matmul(
                ps, lhsT=ctxt16s[hp][:, qloc * 128:(qloc + 1) * 128],
                rhs=wo_sb[:, hp, dw * 512:(dw + 1) * 512],
                start=(hp == 0), stop=(hp == HP - 1),
            )
        osb = osbp.tile([128, 512], F32, tag="osb")
        nc.vector.tensor_copy(osb, ps)
        nc.gpsimd.dma_start(
            out_d[qb * 128:(qb + 1) * 128, dw * 512:(dw + 1) * 512], osb)


def build_nc():
    nc = bacc.Bacc("TRN2", target_bir_lowering=False)
    x_d = nc.dram_tensor("x", [T, D], F32, kind="ExternalInput")
    wq_d = nc.dram_tensor("wq", [D, CD], F32, kind="ExternalInput")
    wk_d = nc.dram_tensor("wk", [D, CD], F32, kind="ExternalInput")
    wv_d = nc.dram_tensor("wv", [D, CD], F32, kind="ExternalInput")
    wo_d = nc.dram_tensor("wo", [CD, D], F32, kind="ExternalInput")
    sel_d = nc.dram_tensor("sel", [2, 128], BF16, kind="ExternalInput")
    out_d = nc.dram_tensor("out", [T, D], F32, kind="ExternalOutput")
    xb16_d = nc.dram_tensor("xb16", [T, D], BF16)  # internal scratch

    with tile.TileContext(nc) as tc:
        with (
            tc.tile_pool(name="consts", bufs=1) as consts,
            tc.tile_pool(name="wsb", bufs=1) as wsb,
            tc.tile_pool(name="wstage", bufs=1) as wstage,
            tc.tile_pool(name="xstage", bufs=2) as xstage,
            tc.tile_pool(name="x16", bufs=2) as x16p,
            tc.tile_pool(name="big", bufs=1) as big,
            tc.tile_pool(name="pt", bufs=2) as ptp,
            tc.tile_pool(name="lv", bufs=4) as lvp,
            tc.tile_pool(name="ctxt16", bufs=8) as ctxt16p,
            tc.tile_pool(name="osb", bufs=2) as osbp,
            tc.tile_pool(name="mm", bufs=2, space="PSUM") as mmp,
            tc.tile_pool(name="st", bufs=2, space="PSUM") as stp,
            tc.tile_pool(name="cx", bufs=1, space="PSUM") as cxp,
        ):
            utri01 = consts.tile([128, 128], BF16, tag="utri01")
            make_upper_triangular(nc, utri01, val=1.0, diag=True)
            sel2 = consts.tile([2, 128], BF16, tag="sel2")
            nc.sync.dma_start(sel2, sel_d[:, :])

            wq_sb = wsb.tile([128, PO, CD], BF16, tag="wq")
            wk_sb = wsb.tile([128, PO, CD], BF16, tag="wk")
            wv_sb = wsb.tile([128, PO, CD], BF16, tag="wv")
            wo_sb = wsb.tile([128, FC, D], BF16, tag="wo")
            for i, (dram, sb, shp) in enumerate((
                (wk_d, wk_sb, (PO, CD)),
                (wq_d, wq_sb, (PO, CD)),
                (wv_d, wv_sb, (PO, CD)),
                (wo_d, wo_sb, (FC, D)),
            )):
                stg = wstage.tile([128, shp[0], shp[1]], F32, tag="ws")
                nc.gpsimd.dma_start(stg, dram.rearrange("(po p) n -> p po n", p=128))
                nc.vector.tensor_copy(sb, stg)

            xt_sb = big.tile([128, PO, T], BF16, tag="xt")
            kt_sb = big.tile([128, HP, T], BF16, tag="kt")
            qt_sb = big.tile([128, HP, T], BF16, tag="qt")
            v_sb = big.tile([128, NB, HL, 65], BF16, tag="v")
            nc.gpsimd.memset(v_sb[:, :, :, 64:65], 1.0)

            for tb in range(NB):
                _emit_xt_tb(nc, tb, x_d, xb16_d, xstage, x16p)

            for tw in range(4):
                for po in range(PO):
                    nc.sync.dma_start_transpose(
                        xt_sb[:, po, tw * 512:(tw + 1) * 512],
                        xb16_d[tw * 512:(tw + 1) * 512,
                               po * 128:(po + 1) * 128])
                _emit_proj_tw(nc, tw, xt_sb, mmp,
                              wq_sb, wk_sb, wv_sb, kt_sb, qt_sb, v_sb)
                qb0 = 4 * tw
                ctxt16s = [ctxt16p.tile([128, 512], BF16, tag="c16",
                                        name=f"c16_{hp}")
                           for hp in range(HP)]
                _emit_attention_quad(nc, qb0, kt_sb, qt_sb, v_sb, utri01,
                                     sel2, ptp, stp, cxp, lvp, mmp, ctxt16s)
                for qloc in range(QUAD):
                    _emit_out_proj_qb(nc, qb0 + qloc, ctxt16s, wo_sb,
                                      mmp, osbp, out_d)

    nc.compile()
    return nc


_CACHE = {}


def _get_nc():
    if "nc" not in _CACHE:
        _CACHE["nc"] = build_nc()
    return _CACHE["nc"]


def make_in_maps(x, Wq, Wk, Wv, Wo):
    x = np.asarray(x, np.float32)
    Wq = np.asarray(Wq, np.float32)
    Wk = np.asarray(Wk, np.float32)
    Wv = np.asarray(Wv, np.float32)
    Wo = np.asarray(Wo, np.float32)
    sel = np.zeros((2, 128), np.float32)
    sel[0, 0:64] = 1.0
    sel[1, 64:128] = 1.0
    sel = sel.astype(ml_dtypes.bfloat16)
    in_maps = []
    for c in range(8):
        b, hh = c // 2, c % 2
        cols = slice(hh * CD, (hh + 1) * CD)
        in_maps.append({
            "x": np.ascontiguousarray(x[b]),
            "wq": np.ascontiguousarray(Wq[:, cols]),
            "wk": np.ascontiguousarray(Wk[:, cols]),
            "wv": np.ascontiguousarray(Wv[:, cols]),
            "wo": np.ascontiguousarray(Wo[cols, :]),
            "sel": sel,
        })
    return in_maps


def gather_output(results, bo):
    bo = np.asarray(bo, np.float32)
    out = np.empty((B, T, D), np.float32)
    for b in range(B):
        out[b] = results[2 * b]["out"] + results[2 * b + 1]["out"] + bo[None, :]
    return out


def kernel(x, Wq, Wk, Wv, Wo, bo):
    nc = _get_nc()
    in_maps = make_in_maps(x, Wq, Wk, Wv, Wo)
    res = run_bass_kernel_spmd(nc, in_maps, core_ids=list(range(8)))
    return gather_output(res.results, bo)


# revision 24
# speedup vs baseline: 1.2457x; 1.0283x over previous
"""Multi-head causal self-attention (B=4, T=2048, D=1024, H=16) on 8 TRN2 cores.

Sharding (hardcoded): data-parallel over the 4 batches x tensor-parallel over
head halves. Core c handles batch c//2 and local heads (c%2)*8 .. (c%2)*8+7
for all 2048 positions. Every core runs the same SPMD program on its slice:

  x[b] [2048,1024] -> bf16 -> x^T in SBUF via SBUF->SBUF DMA transposes
  Q^T = (Wq_slice)^T x^T,  K^T = (Wk_slice)^T x^T  (dh-pairs packed on 128
  partitions), V = x Wv_slice (+ ones column for the softmax denominator)
  S^T = K Q^T per 128x512 block; the two heads of a pair run as concurrent
  row-group matmuls (tile_position (0,0)/(64,0)) into one [128,1024] psum
  tile; diagonal blocks stream only the causal columns,
  P^T = exp(S^T / 8) (one ScalarE activation per key block covering both
  parities, bf16 out), strict-upper within-block masking via utri mul,
  ctx^T = V^T P per 4-key-block segment (V stationary; row 64 = denom l),
  normalize: 1/l broadcast via a K=2 selector matmul on the PE,
  partial_out = ctx^T.T @ Wo_slice.

The host sums the two partial outputs per batch and adds the bias bo.
"""
import ml_dtypes
import numpy as np

import concourse.bass as bass
import concourse.mybir as mybir
import concourse.tile as tile
from concourse import bacc
from concourse.bass_utils import run_bass_kernel_spmd
from concourse.masks import make_upper_triangular

F32 = mybir.dt.float32
BF16 = mybir.dt.bfloat16
AF = mybir.ActivationFunctionType

B, T, D = 4, 2048, 1024
HL = 8              # local heads per core
HP = HL // 2        # local head pairs (two heads share 128 partitions)
DH = 64
PO = D // 128       # contraction chunks over D
CD = HL * DH        # 512: local context feature dim
FC = CD // 128      # 4
NB = T // 128       # 16 query/key blocks of 128
QUAD = 4            # query blocks handled together (512 S^T columns)
SCALE = 1.0 / 8.0   # 1/sqrt(DH)
SEG = 4             # key blocks per P^T segment (AV granularity)


def _emit_xt_tb(nc, tb, x_d, xb16_d, xstage, x16p):
    """One 128-row block of x: f32 load (scalar q), bf16 cast (DVE), store
    to DRAM scratch (gpsimd swdge q)."""
    xf = xstage.tile([128, D], F32, tag="xf")
    nc.scalar.dma_start(xf, x_d[tb * 128:(tb + 1) * 128, :])
    x16 = x16p.tile([128, D], BF16, tag="x16")
    nc.vector.tensor_copy(x16, xf)
    nc.gpsimd.dma_start(xb16_d[tb * 128:(tb + 1) * 128, :], x16)


def _emit_proj_tw(nc, tw, xt_sb, mmp, wq_sb, wk_sb, wv_sb, kt_sb, qt_sb, v_sb):
    """Project K^T, Q^T, V for one 512-column group of x^T."""
    tsl = slice(tw * 512, (tw + 1) * 512)
    for hp in range(HP):
        ps = mmp.tile([128, 512], F32, tag="mm")
        for po in range(PO):
            nc.tensor.matmul(
                ps, lhsT=wk_sb[:, po, hp * 128:(hp + 1) * 128], rhs=xt_sb[:, po, tsl],
                start=(po == 0), stop=(po == PO - 1),
            )
        nc.vector.tensor_copy(kt_sb[:, hp, tsl], ps)
    for hp in range(HP):
        ps = mmp.tile([128, 512], F32, tag="mm")
        for po in range(PO):
            nc.tensor.matmul(
                ps, lhsT=wq_sb[:, po, hp * 128:(hp + 1) * 128], rhs=xt_sb[:, po, tsl],
                start=(po == 0), stop=(po == PO - 1),
            )
        nc.vector.tensor_copy(qt_sb[:, hp, tsl], ps)
    for tb in range(4):
        kb = tw * 4 + tb
        ps = mmp.tile([128, 512], F32, tag="mm")
        for po in range(PO):
            nc.tensor.matmul(
                ps, lhsT=xt_sb[:, po, kb * 128:(kb + 1) * 128], rhs=wv_sb[:, po, :],
                start=(po == 0), stop=(po == PO - 1),
            )
        nc.vector.tensor_copy(
            v_sb[:, kb, :, 0:64], ps.rearrange("p (h d) -> p h d", h=HL)
        )


def _emit_attention_quad(nc, qb0, kt_sb, qt_sb, v_sb, utri01, sel2,
                         ptp, stp, cxp, lvp, mmp, ctxt16s):
    """Attention for query blocks qb0..qb0+3, all 4 local head pairs.

    For each key block kb one matmul pair (row groups (0,0)/(64,0), shared
    [128,1024] psum tile) covers all four query blocks of both parities.
    Diagonal key blocks stream only columns >= the diagonal; P^T columns
    left of the diagonal are never written nor read.
    """
    nkb = qb0 + QUAD
    nseg = (nkb + SEG - 1) // SEG
    # per-quad softmax denominators: l rows gathered as [64, (hp, par, 8)]
    # so the reciprocal runs 16 elems/lane instead of 512
    lv = lvp.tile([64, HP, 2, 8], BF16, tag="lv")
    cAs, cBs = [], []
    for hp in range(HP):
        ctxt16 = ctxt16s[hp]
        psA = cxp.tile([128, 512], F32, tag="cxA")
        psB = cxp.tile([128, 512], F32, tag="cxB")
        for seg in range(nseg):
            s0, s1 = seg * SEG, min(nkb, (seg + 1) * SEG)
            pt = ptp.tile([128, SEG, 2, 512], BF16, tag="pt")
            for kb in range(s0, s1):
                st = stp.tile([128, 1024], F32, tag="st")
                j = max(0, kb - qb0)  # first valid 128-col of the quad
                for par, tp in ((0, (0, 0)), (1, (64, 0))):
                    lo = 64 * par
                    nc.tensor.matmul(
                        st[:, par * 512 + j * 128:(par + 1) * 512],
                        lhsT=kt_sb[lo:lo + 64, hp, kb * 128:(kb + 1) * 128],
                        rhs=qt_sb[lo:lo + 64, hp,
                                  (qb0 + j) * 128:(qb0 + QUAD) * 128],
                        start=True, stop=True, tile_position=tp,
                    )
                # exp of both parities in one ScalarE instruction, only the
                # causal columns of the quad
                st_v = st.rearrange("p (a c) -> p a c", a=2)
                nc.scalar.activation(pt[:, kb - s0, :, j * 128:512],
                                     st_v[:, :, j * 128:512], AF.Exp,
                                     scale=SCALE)
                if kb >= qb0:
                    for par in range(2):
                        nc.vector.tensor_mul(
                            pt[:, kb - s0, par, j * 128:(j + 1) * 128],
                            pt[:, kb - s0, par, j * 128:(j + 1) * 128], utri01)
            # AV for this segment (accumulating across segments)
            for par, ps in ((0, psA), (1, psB)):
                for kb in range(s0, s1):
                    j = max(0, kb - qb0)
                    nc.tensor.matmul(
                        ps[0:65, j * 128:512],
                        lhsT=v_sb[:, kb, 2 * hp + par, :],
                        rhs=pt[:, kb - s0, par, j * 128:512],
                        start=(kb == 0), stop=(kb == nkb - 1),
                    )
        # psum -> sbuf copies (free the cx banks), l rows into the quad
        # gather, par1 ctx partition-shifted to 64:128 early
        cA = lvp.tile([65, 512], BF16, tag="cA", name=f"cA{hp}")
        nc.vector.tensor_copy(cA, psA[0:65, :])
        cB = lvp.tile([65, 512], BF16, tag="cB", name=f"cB{hp}")
        nc.vector.tensor_copy(cB, psB[0:65, :])
        nc.sync.dma_start(ctxt16[64:128, :], cB[0:64, :])
        nc.sync.dma_start(lv[:, hp, 0, :], cA[64:65, :])
        nc.sync.dma_start(lv[:, hp, 1, :], cB[64:65, :])
        cAs.append(cA)
        cBs.append(cB)
    # one reciprocal for the whole quad (16 elems/lane), scatter to the
    # [2, hp, q] layout the selector matmul streams from
    lvi = lvp.tile([64, HP, 2, 8], F32, tag="lvi")
    nc.vector.reciprocal(lvi, lv)
    lvi16 = lvp.tile([64, HP, 2, 8], BF16, tag="lvi16")
    nc.vector.tensor_copy(lvi16, lvi)
    llin = lvp.tile([2, HP, 64, 8], BF16, tag="llin")
    for par in range(2):
        for h in range(HP):
            nc.sync.dma_start(llin[par:par + 1, h, :, :],
                              lvi16[:, h, par, :])
    for hp in range(HP):
        ctxt16 = ctxt16s[hp]
        bc = mmp.tile([128, 512], F32, tag="mm")
        nc.tensor.matmul(bc, lhsT=sel2, rhs=llin[:, hp, :, :],
                         start=True, stop=True)
        nc.vector.tensor_mul(ctxt16[0:64, :], cAs[hp][0:64, :], bc[0:64, :])
        nc.vector.tensor_mul(ctxt16[64:128, :], ctxt16[64:128, :],
                             bc[64:128, :])


def _emit_out_proj_qb(nc, qb, ctxt16s, wo_sb, mmp, osbp, out_d):
    """Output projection for query block qb (reads all 4 head pairs)."""
    qloc = qb % QUAD
    for dw in range(2):
        ps = mmp.tile([128, 512], F32, tag="mm")
        for hp in range(HP):
            nc.tensor.matmul(
                ps, lhsT=ctxt16s[hp][:, qloc * 128:(qloc + 1) * 128],
                rhs=wo_sb[:, hp, dw * 512:(dw + 1) * 512],
                start=(hp == 0), stop=(hp == HP - 1),
            )
        osb = osbp.tile([128, 512], F32, tag="osb")
        nc.vector.tensor_copy(osb, ps)
        eng = nc.sync if dw == 0 else nc.scalar
        eng.dma_start(
            out_d[qb * 128:(qb + 1) * 128, dw * 512:(dw + 1) * 512], osb)


def build_nc():
    nc = bacc.Bacc("TRN2", target_bir_lowering=False)
    x_d = nc.dram_tensor("x", [T, D], F32, kind="ExternalInput")
    wq_d = nc.dram_tensor("wq", [D, CD], F32, kind="ExternalInput")
    wk_d = nc.dram_tensor("wk", [D, CD], F32, kind="ExternalInput")
    wv_d = nc.dram_tensor("wv", [D, CD], F32, kind="ExternalInput")
    wo_d = nc.dram_tensor("wo", [CD, D], F32, kind="ExternalInput")
    sel_d = nc.dram_tensor("sel", [2, 128], BF16, kind="ExternalInput")
    out_d = nc.dram_tensor("out", [T, D], F32, kind="ExternalOutput")
    xb16_d = nc.dram_tensor("xb16", [T, D], BF16)  # internal scratch

    with tile.TileContext(nc) as tc:
        with (
            tc.tile_pool(name="consts", bufs=1) as consts,
            tc.tile_pool(name="wsb", bufs=1) as wsb,
            tc.tile_pool(name="wstage", bufs=2) as wstage,
            tc.tile_pool(name="xstage", bufs=2) as xstage,
            tc.tile_pool(name="x16", bufs=2) as x16p,
            tc.tile_pool(name="big", bufs=1) as big,
            tc.tile_pool(name="pt", bufs=2) as ptp,
            tc.tile_pool(name="lv", bufs=4) as lvp,
            tc.tile_pool(name="ctxt16", bufs=8) as ctxt16p,
            tc.tile_pool(name="osb", bufs=2) as osbp,
            tc.tile_pool(name="mm", bufs=2, space="PSUM") as mmp,
            tc.tile_pool(name="st", bufs=2, space="PSUM") as stp,
            tc.tile_pool(name="cx", bufs=1, space="PSUM") as cxp,
        ):
            utri01 = consts.tile([128, 128], BF16, tag="utri01")
            make_upper_triangular(nc, utri01, val=1.0, diag=True)
            sel2 = consts.tile([2, 128], BF16, tag="sel2")
            nc.sync.dma_start(sel2, sel_d[:, :])

            wq_sb = wsb.tile([128, PO, CD], BF16, tag="wq")
            wk_sb = wsb.tile([128, PO, CD], BF16, tag="wk")
            wv_sb = wsb.tile([128, PO, CD], BF16, tag="wv")
            wo_sb = wsb.tile([128, FC, D], BF16, tag="wo")
            for i, (dram, sb, shp, eng) in enumerate((
                (wk_d, wk_sb, (PO, CD), nc.sync),
                (wq_d, wq_sb, (PO, CD), nc.sync),
                (wv_d, wv_sb, (PO, CD), nc.gpsimd),
                (wo_d, wo_sb, (FC, D), nc.gpsimd),
            )):
                dview = dram.rearrange("(po p) n -> p po n", p=128)
                for h in range(2):
                    stg = wstage.tile([128, 4, 512], F32, tag="ws")
                    if shp[0] == PO:  # wk/wq/wv: halve along po
                        src = dview[:, h * 4:(h + 1) * 4, :]
                        dst = sb[:, h * 4:(h + 1) * 4, :]
                    else:             # wo: halve along n
                        src = dview[:, :, h * 512:(h + 1) * 512]
                        dst = sb[:, :, h * 512:(h + 1) * 512]
                    eng.dma_start(stg, src)
                    nc.vector.tensor_copy(dst, stg)

            xt_sb = big.tile([128, PO, T], BF16, tag="xt")
            kt_sb = big.tile([128, HP, T], BF16, tag="kt")
            qt_sb = big.tile([128, HP, T], BF16, tag="qt")
            v_sb = big.tile([128, NB, HL, 65], BF16, tag="v")
            nc.gpsimd.memset(v_sb[:, :, :, 64:65], 1.0)

            for tb in range(NB):
                _emit_xt_tb(nc, tb, x_d, xb16_d, xstage, x16p)

            # all x^T transposes upfront, split across the two hwdge queues
            for tw in range(4):
                eng = nc.sync if tw < 2 else nc.scalar
                for po in range(PO):
                    eng.dma_start_transpose(
                        xt_sb[:, po, tw * 512:(tw + 1) * 512],
                        xb16_d[tw * 512:(tw + 1) * 512,
                               po * 128:(po + 1) * 128])

            for tw in range(4):
                _emit_proj_tw(nc, tw, xt_sb, mmp,
                              wq_sb, wk_sb, wv_sb, kt_sb, qt_sb, v_sb)
                qb0 = 4 * tw
                ctxt16s = [ctxt16p.tile([128, 512], BF16, tag="c16",
                                        name=f"c16_{hp}")
                           for hp in range(HP)]
                _emit_attention_quad(nc, qb0, kt_sb, qt_sb, v_sb, utri01,
                                     sel2, ptp, stp, cxp, lvp, mmp, ctxt16s)
                for qloc in range(QUAD):
                    _emit_out_proj_qb(nc, qb0 + qloc, ctxt16s, wo_sb,
                                      mmp, osbp, out_d)

    nc.compile()
    return nc


_CACHE = {}


def _get_nc():
    if "nc" not in _CACHE:
        _CACHE["nc"] = build_nc()
    return _CACHE["nc"]


def make_in_maps(x, Wq, Wk, Wv, Wo):
    x = np.asarray(x, np.float32)
    Wq = np.asarray(Wq, np.float32)
    Wk = np.asarray(Wk, np.float32)
    Wv = np.asarray(Wv, np.float32)
    Wo = np.asarray(Wo, np.float32)
    sel = np.zeros((2, 128), np.float32)
    sel[0, 0:64] = 1.0
    sel[1, 64:128] = 1.0
    sel = sel.astype(ml_dtypes.bfloat16)
    in_maps = []
    for c in range(8):
        b, hh = c // 2, c % 2
        cols = slice(hh * CD, (hh + 1) * CD)
        in_maps.append({
            "x": np.ascontiguousarray(x[b]),
            "wq": np.ascontiguousarray(Wq[:, cols]),
            "wk": np.ascontiguousarray(Wk[:, cols]),
            "wv": np.ascontiguousarray(Wv[:, cols]),
            "wo": np.ascontiguousarray(Wo[cols, :]),
            "sel": sel,
        })
    return in_maps


def gather_output(results, bo):
    bo = np.asarray(bo, np.float32)
    out = np.empty((B, T, D), np.float32)
    for b in range(B):
        out[b] = results[2 * b]["out"] + results[2 * b + 1]["out"] + bo[None, :]
    return out


def kernel(x, Wq, Wk, Wv, Wo, bo):
    nc = _get_nc()
    in_maps = make_in_maps(x, Wq, Wk, Wv, Wo)
    res = run_bass_kernel_spmd(nc, in_maps, core_ids=list(range(8)))
    return gather_output(res.results, bo)


# revision 26
# speedup vs baseline: 1.4914x; 1.1973x over previous
"""Multi-head causal self-attention (B=4, T=2048, D=1024, H=16) on 8 TRN2 cores.

Sharding (hardcoded): data-parallel over the 4 batches x tensor-parallel over
head halves. Core c handles batch c//2 and local heads (c%2)*8 .. (c%2)*8+7
for all 2048 positions. Every core runs the same SPMD program on its slice:

  x[b] [2048,1024] -> bf16 -> x^T in SBUF via SBUF->SBUF DMA transposes
  Q^T = (Wq_slice)^T x^T,  K^T = (Wk_slice)^T x^T  (dh-pairs packed on 128
  partitions), V = x Wv_slice (+ ones column for the softmax denominator)
  S^T = K Q^T per 128x512 block; the two heads of a pair run as concurrent
  row-group matmuls (tile_position (0,0)/(64,0)) into one [128,1024] psum
  tile; diagonal blocks stream only the causal columns,
  P^T = exp(S^T / 8) (one ScalarE activation per key block covering both
  parities, bf16 out), strict-upper within-block masking via utri mul,
  ctx^T = V^T P per 4-key-block segment (V stationary; row 64 = denom l),
  normalize: 1/l broadcast via a K=2 selector matmul on the PE,
  partial_out = ctx^T.T @ Wo_slice.

The host sums the two partial outputs per batch and adds the bias bo.
"""
import ml_dtypes
import numpy as np

import concourse.bass as bass
import concourse.mybir as mybir
import concourse.tile as tile
from concourse import bacc
from concourse.bass_utils import run_bass_kernel_spmd
from concourse.masks import make_upper_triangular

F32 = mybir.dt.float32
BF16 = mybir.dt.bfloat16
AF = mybir.ActivationFunctionType

B, T, D = 4, 2048, 1024
HL = 8              # local heads per core
HP = HL // 2        # local head pairs (two heads share 128 partitions)
DH = 64
PO = D // 128       # contraction chunks over D
CD = HL * DH        # 512: local context feature dim
FC = CD // 128      # 4
NB = T // 128       # 16 query/key blocks of 128
QUAD = 4            # query blocks handled together (512 S^T columns)
SCALE = 1.0 / 8.0   # 1/sqrt(DH)
SEG = 4             # key blocks per P^T segment (AV granularity)


def _emit_xt_tb(nc, tb, x_d, xb16_d, xstage, x16p):
    """One 128-row block of x: f32 load (scalar q), bf16 cast (DVE), store
    to DRAM scratch (gpsimd swdge q)."""
    xf = xstage.tile([128, D], F32, tag="xf")
    nc.scalar.dma_start(xf, x_d[tb * 128:(tb + 1) * 128, :])
    x16 = x16p.tile([128, D], BF16, tag="x16")
    nc.vector.tensor_copy(x16, xf)
    nc.gpsimd.dma_start(xb16_d[tb * 128:(tb + 1) * 128, :], x16)


def _emit_proj_tw(nc, tw, xt_sb, mmp, wq_sb, wk_sb, wv_sb, kt_sb, qt_sb, v_sb):
    """Project K^T, Q^T, V for one 512-column group of x^T."""
    tsl = slice(tw * 512, (tw + 1) * 512)
    for hp in range(HP):
        ps = mmp.tile([128, 512], F32, tag="mm")
        for po in range(PO):
            nc.tensor.matmul(
                ps, lhsT=wk_sb[:, po, hp * 128:(hp + 1) * 128], rhs=xt_sb[:, po, tsl],
                start=(po == 0), stop=(po == PO - 1),
            )
        nc.vector.tensor_copy(kt_sb[:, hp, tsl], ps)
    for hp in range(HP):
        ps = mmp.tile([128, 512], F32, tag="mm")
        for po in range(PO):
            nc.tensor.matmul(
                ps, lhsT=wq_sb[:, po, hp * 128:(hp + 1) * 128], rhs=xt_sb[:, po, tsl],
                start=(po == 0), stop=(po == PO - 1),
            )
        nc.vector.tensor_copy(qt_sb[:, hp, tsl], ps)
    for tb in range(4):
        kb = tw * 4 + tb
        ps = mmp.tile([128, 512], F32, tag="mm")
        for po in range(PO):
            nc.tensor.matmul(
                ps, lhsT=xt_sb[:, po, kb * 128:(kb + 1) * 128], rhs=wv_sb[:, po, :],
                start=(po == 0), stop=(po == PO - 1),
            )
        nc.vector.tensor_copy(
            v_sb[:, kb, :, 0:64], ps.rearrange("p (h d) -> p h d", h=HL)
        )


def _emit_attention_quad(nc, qb0, kt_sb, qt_sb, v_sb, utri01, sel2,
                         ptp, stp, cxp, lvp, mmp, ctxt16s):
    """Attention for query blocks qb0..qb0+3, all 4 local head pairs.

    For each key block kb one matmul pair (row groups (0,0)/(64,0), shared
    [128,1024] psum tile) covers all four query blocks of both parities.
    Diagonal key blocks stream only columns >= the diagonal; P^T columns
    left of the diagonal are never written nor read.
    """
    nkb = qb0 + QUAD
    nseg = (nkb + SEG - 1) // SEG
    # per-quad softmax denominators: l rows gathered as [64, (hp, par, 8)]
    # so the reciprocal runs 16 elems/lane instead of 512
    lv = lvp.tile([64, HP, 2, 8], BF16, tag="lv")
    cAs, cBs = [], []
    for hp in range(HP):
        ctxt16 = ctxt16s[hp]
        psA = cxp.tile([128, 512], F32, tag="cxA")
        psB = cxp.tile([128, 512], F32, tag="cxB")
        for seg in range(nseg):
            s0, s1 = seg * SEG, min(nkb, (seg + 1) * SEG)
            pt = ptp.tile([128, SEG, 2, 512], BF16, tag="pt")
            for kb in range(s0, s1):
                st = stp.tile([128, 1024], F32, tag="st")
                j = max(0, kb - qb0)  # first valid 128-col of the quad
                for par, tp in ((0, (0, 0)), (1, (64, 0))):
                    lo = 64 * par
                    nc.tensor.matmul(
                        st[:, par * 512 + j * 128:(par + 1) * 512],
                        lhsT=kt_sb[lo:lo + 64, hp, kb * 128:(kb + 1) * 128],
                        rhs=qt_sb[lo:lo + 64, hp,
                                  (qb0 + j) * 128:(qb0 + QUAD) * 128],
                        start=True, stop=True, tile_position=tp,
                    )
                # exp of both parities in one ScalarE instruction, only the
                # causal columns of the quad
                st_v = st.rearrange("p (a c) -> p a c", a=2)
                nc.scalar.activation(pt[:, kb - s0, :, j * 128:512],
                                     st_v[:, :, j * 128:512], AF.Exp,
                                     scale=SCALE)
                if kb >= qb0:
                    for par in range(2):
                        nc.vector.tensor_mul(
                            pt[:, kb - s0, par, j * 128:(j + 1) * 128],
                            pt[:, kb - s0, par, j * 128:(j + 1) * 128], utri01)
            # AV for this segment (accumulating across segments)
            for par, ps in ((0, psA), (1, psB)):
                for kb in range(s0, s1):
                    j = max(0, kb - qb0)
                    nc.tensor.matmul(
                        ps[0:65, j * 128:512],
                        lhsT=v_sb[:, kb, 2 * hp + par, :],
                        rhs=pt[:, kb - s0, par, j * 128:512],
                        start=(kb == 0), stop=(kb == nkb - 1),
                    )
        # psum -> sbuf copies (free the cx banks), l rows into the quad
        # gather, par1 ctx partition-shifted to 64:128 early
        cA = lvp.tile([65, 512], BF16, tag="cA", name=f"cA{hp}")
        nc.vector.tensor_copy(cA, psA[0:65, :])
        cB = lvp.tile([65, 512], BF16, tag="cB", name=f"cB{hp}")
        nc.vector.tensor_copy(cB, psB[0:65, :])
        nc.sync.dma_start(ctxt16[64:128, :], cB[0:64, :])
        nc.sync.dma_start(lv[:, hp, 0, :], cA[64:65, :])
        nc.sync.dma_start(lv[:, hp, 1, :], cB[64:65, :])
        cAs.append(cA)
        cBs.append(cB)
    # one reciprocal for the whole quad (16 elems/lane), scatter to the
    # [2, hp, q] layout the selector matmul streams from
    lvi = lvp.tile([64, HP, 2, 8], F32, tag="lvi")
    nc.vector.reciprocal(lvi, lv)
    lvi16 = lvp.tile([64, HP, 2, 8], BF16, tag="lvi16")
    nc.vector.tensor_copy(lvi16, lvi)
    llin = lvp.tile([2, HP, 64, 8], BF16, tag="llin")
    for par in range(2):
        for h in range(HP):
            nc.sync.dma_start(llin[par:par + 1, h, :, :],
                              lvi16[:, h, par, :])
    return cAs, llin


def _emit_quad_norm(nc, sel2, cAs, llin, mmp, ctxt16s):
    """PE broadcast of 1/l + the normalize multiplies (phase B; emitted
    after the next tw's projections so their matmuls cover the exp tail)."""
    for hp in range(HP):
        ctxt16 = ctxt16s[hp]
        bc = mmp.tile([128, 512], F32, tag="mm")
        nc.tensor.matmul(bc, lhsT=sel2, rhs=llin[:, hp, :, :],
                         start=True, stop=True)
        nc.vector.tensor_mul(ctxt16[0:64, :], cAs[hp][0:64, :], bc[0:64, :])
        nc.vector.tensor_mul(ctxt16[64:128, :], ctxt16[64:128, :],
                             bc[64:128, :])


def _emit_out_proj_qb(nc, qb, ctxt16s, wo_sb, mmp, osbp, out_d):
    """Output projection for query block qb (reads all 4 head pairs)."""
    qloc = qb % QUAD
    for dw in range(2):
        ps = mmp.tile([128, 512], F32, tag="mm")
        for hp in range(HP):
            nc.tensor.matmul(
                ps, lhsT=ctxt16s[hp][:, qloc * 128:(qloc + 1) * 128],
                rhs=wo_sb[:, hp, dw * 512:(dw + 1) * 512],
                start=(hp == 0), stop=(hp == HP - 1),
            )
        osb = osbp.tile([128, 512], F32, tag="osb")
        nc.vector.tensor_copy(osb, ps)
        eng = nc.sync if dw == 0 else nc.scalar
        eng.dma_start(
            out_d[qb * 128:(qb + 1) * 128, dw * 512:(dw + 1) * 512], osb)


def build_nc():
    nc = bacc.Bacc("TRN2", target_bir_lowering=False)
    x_d = nc.dram_tensor("x", [T, D], F32, kind="ExternalInput")
    wq_d = nc.dram_tensor("wq", [D, CD], F32, kind="ExternalInput")
    wk_d = nc.dram_tensor("wk", [D, CD], F32, kind="ExternalInput")
    wv_d = nc.dram_tensor("wv", [D, CD], F32, kind="ExternalInput")
    wo_d = nc.dram_tensor("wo", [CD, D], F32, kind="ExternalInput")
    sel_d = nc.dram_tensor("sel", [2, 128], BF16, kind="ExternalInput")
    out_d = nc.dram_tensor("out", [T, D], F32, kind="ExternalOutput")
    xb16_d = nc.dram_tensor("xb16", [T, D], BF16)  # internal scratch

    with tile.TileContext(nc) as tc:
        with (
            tc.tile_pool(name="consts", bufs=1) as consts,
            tc.tile_pool(name="wsb", bufs=1) as wsb,
            tc.tile_pool(name="wstage", bufs=2) as wstage,
            tc.tile_pool(name="xstage", bufs=2) as xstage,
            tc.tile_pool(name="x16", bufs=2) as x16p,
            tc.tile_pool(name="big", bufs=1) as big,
            tc.tile_pool(name="pt", bufs=2) as ptp,
            tc.tile_pool(name="lv", bufs=4) as lvp,
            tc.tile_pool(name="ctxt16", bufs=8) as ctxt16p,
            tc.tile_pool(name="osb", bufs=2) as osbp,
            tc.tile_pool(name="mm", bufs=2, space="PSUM") as mmp,
            tc.tile_pool(name="st", bufs=2, space="PSUM") as stp,
            tc.tile_pool(name="cx", bufs=1, space="PSUM") as cxp,
        ):
            utri01 = consts.tile([128, 128], BF16, tag="utri01")
            make_upper_triangular(nc, utri01, val=1.0, diag=True)
            sel2 = consts.tile([2, 128], BF16, tag="sel2")
            nc.sync.dma_start(sel2, sel_d[:, :])

            wq_sb = wsb.tile([128, PO, CD], BF16, tag="wq")
            wk_sb = wsb.tile([128, PO, CD], BF16, tag="wk")
            wv_sb = wsb.tile([128, PO, CD], BF16, tag="wv")
            wo_sb = wsb.tile([128, FC, D], BF16, tag="wo")
            for i, (dram, sb, shp, eng) in enumerate((
                (wk_d, wk_sb, (PO, CD), nc.sync),
                (wq_d, wq_sb, (PO, CD), nc.sync),
                (wv_d, wv_sb, (PO, CD), nc.gpsimd),
                (wo_d, wo_sb, (FC, D), nc.gpsimd),
            )):
                dview = dram.rearrange("(po p) n -> p po n", p=128)
                for h in range(2):
                    stg = wstage.tile([128, 4, 512], F32, tag="ws")
                    if shp[0] == PO:  # wk/wq/wv: halve along po
                        src = dview[:, h * 4:(h + 1) * 4, :]
                        dst = sb[:, h * 4:(h + 1) * 4, :]
                    else:             # wo: halve along n
                        src = dview[:, :, h * 512:(h + 1) * 512]
                        dst = sb[:, :, h * 512:(h + 1) * 512]
                    eng.dma_start(stg, src)
                    nc.vector.tensor_copy(dst, stg)

            xt_sb = big.tile([128, PO, T], BF16, tag="xt")
            kt_sb = big.tile([128, HP, T], BF16, tag="kt")
            qt_sb = big.tile([128, HP, T], BF16, tag="qt")
            v_sb = big.tile([128, NB, HL, 65], BF16, tag="v")
            nc.gpsimd.memset(v_sb[:, :, :, 64:65], 1.0)

            for tb in range(NB):
                _emit_xt_tb(nc, tb, x_d, xb16_d, xstage, x16p)

            # all x^T transposes upfront, split across the two hwdge queues
            for tw in range(4):
                eng = nc.sync if tw < 2 else nc.scalar
                for po in range(PO):
                    eng.dma_start_transpose(
                        xt_sb[:, po, tw * 512:(tw + 1) * 512],
                        xb16_d[tw * 512:(tw + 1) * 512,
                               po * 128:(po + 1) * 128])

            _emit_proj_tw(nc, 0, xt_sb, mmp,
                          wq_sb, wk_sb, wv_sb, kt_sb, qt_sb, v_sb)
            for tw in range(4):
                qb0 = 4 * tw
                ctxt16s = [ctxt16p.tile([128, 512], BF16, tag="c16",
                                        name=f"c16_{hp}")
                           for hp in range(HP)]
                cAs, llin = _emit_attention_quad(
                    nc, qb0, kt_sb, qt_sb, v_sb, utri01,
                    sel2, ptp, stp, cxp, lvp, mmp, ctxt16s)
                if tw < 3:
                    # next tw's projections run on the PE while this quad's
                    # exp tail + 1/l chain completes
                    _emit_proj_tw(nc, tw + 1, xt_sb, mmp,
                                  wq_sb, wk_sb, wv_sb, kt_sb, qt_sb, v_sb)
                _emit_quad_norm(nc, sel2, cAs, llin, mmp, ctxt16s)
                for qloc in range(QUAD):
                    _emit_out_proj_qb(nc, qb0 + qloc, ctxt16s, wo_sb,
                                      mmp, osbp, out_d)

    nc.compile()
    return nc


_CACHE = {}


def _get_nc():
    if "nc" not in _CACHE:
        _CACHE["nc"] = build_nc()
    return _CACHE["nc"]


def make_in_maps(x, Wq, Wk, Wv, Wo):
    x = np.asarray(x, np.float32)
    Wq = np.asarray(Wq, np.float32)
    Wk = np.asarray(Wk, np.float32)
    Wv = np.asarray(Wv, np.float32)
    Wo = np.asarray(Wo, np.float32)
    sel = np.zeros((2, 128), np.float32)
    sel[0, 0:64] = 1.0
    sel[1, 64:128] = 1.0
    sel = sel.astype(ml_dtypes.bfloat16)
    in_maps = []
    for c in range(8):
        b, hh = c // 2, c % 2
        cols = slice(hh * CD, (hh + 1) * CD)
        in_maps.append({
            "x": np.ascontiguousarray(x[b]),
            "wq": np.ascontiguousarray(Wq[:, cols]),
            "wk": np.ascontiguousarray(Wk[:, cols]),
            "wv": np.ascontiguousarray(Wv[:, cols]),
            "wo": np.ascontiguousarray(Wo[cols, :]),
            "sel": sel,
        })
    return in_maps


def gather_output(results, bo):
    bo = np.asarray(bo, np.float32)
    out = np.empty((B, T, D), np.float32)
    for b in range(B):
        out[b] = results[2 * b]["out"] + results[2 * b + 1]["out"] + bo[None, :]
    return out


def kernel(x, Wq, Wk, Wv, Wo, bo):
    nc = _get_nc()
    in_maps = make_in_maps(x, Wq, Wk, Wv, Wo)
    res = run_bass_kernel_spmd(nc, in_maps, core_ids=list(range(8)))
    return gather_output(res.results, bo)
